# revision 49
# baseline (speedup 1.0000x reference)
"""Trainium2 Bass kernel for nn_MACEConvolutionLayer (v2).

Strategy (8 NeuronCores, no collectives):
  - Edges sharded by destination node range (1250 nodes/core), sorted and
    packed into 10 windows of 128 nodes x 1024 edge slots per core.
  - All data bf16 on device (accumulations in fp32 PSUM).
  - Monomials (per-sample channel outer products) are formed directly in
    TRANSPOSED (feature-major) layout: operands are pre-replicated across
    partitions with tiny 0/1 selection matmuls on the tensor engine
    (R4: K=4 row-replicate x32, RT32: K=32 row-tile x4), then a single DVE
    tensor_tensor (2x bf16 mode) forms each monomial tile in SBUF, ready to
    be the lhsT of the emission matmuls. No per-block PE transposes, no
    PSUM evacuation copies.
  - Edge phase: radial-MLP hidden f2 is kept feature-major; rw3 is folded
    into the A2 bilinear weights so edge monomials come straight from f2.
  - Node phase: aa pass (a x a) u-major monomials, qa pass (q x a) v-major
    monomials (omega rows transposed) so the replicated side is always `a`
    (AX tiles shared by both passes).

Feature layout on device is kappa-major: col(l, i, u) = LOFF[l] + i*32 + u.
"""
import sys, os

sys.path.insert(0, '/opt/trn_rl_repo')

import numpy as np
import ml_dtypes

MUL = 32
DIMS = (1, 3, 5)
HID = 288
N_NODES = 10000
N_EDGES = 64000
RHID = 64
SQM = float(np.sqrt(MUL))
LOFF = [0, 32, 128]
SOFF = [0, 1, 4]
PATHS_FULL = [(0,0,0),(0,1,1),(0,2,2),(1,0,1),(1,1,0),(1,1,2),(1,2,1),(2,0,2),(2,1,1),(2,2,0),(2,2,2)]
O2_UVW = [(0,1,1),(0,2,2),(1,2,1)]
O2_UVU = [(0,0,0),(1,1,0),(1,1,2),(2,2,0),(2,2,2)]

N_CORES = 8
NODES_PER_CORE = 1250
WIN = 128
N_WIN = 10
ESLOT = 1024
E_PAD = N_WIN * ESLOT   # 10240
ET_PER_WIN = ESLOT // 128  # 8
BF = ml_dtypes.bfloat16

COMPS = [(l, i) for l in range(3) for i in range(DIMS[l])]  # 9 irrep components
TCOMP = {c: t for t, c in enumerate(COMPS)}
CCOL = {c: LOFF[c[0]] + c[1] * 32 for c in COMPS}

RUNCAP = 3          # max monomial blocks fused into one DVE op
GAPTOL = 2          # emission position-gap merge tolerance (post-permutation)
GP_MAX_RUNS = 8     # single-block monomial runs assigned to gpsimd
EDGE_GP = True      # edge monomials on gpsimd


def cg_np():
    s2, s3, s5, s6 = map(np.sqrt, (2.0, 3.0, 5.0, 6.0))
    B = np.zeros((5, 3, 3))
    B[0, 0, 1] = B[0, 1, 0] = 1 / s2
    B[1, 1, 2] = B[1, 2, 1] = 1 / s2
    B[2] = np.diag([-1.0, -1.0, 2.0]) / s6
    B[3, 0, 2] = B[3, 2, 0] = 1 / s2
    B[4] = np.diag([1.0, -1.0, 0.0]) / s2
    C = {}
    C[(0, 0, 0)] = np.ones((1, 1, 1))
    C[(0, 1, 1)] = (np.eye(3) / s3)[None]
    C[(1, 0, 1)] = np.transpose(C[(0, 1, 1)], (1, 0, 2))
    C[(0, 2, 2)] = (np.eye(5) / s5)[None]
    C[(2, 0, 2)] = np.transpose(C[(0, 2, 2)], (1, 0, 2))
    C[(1, 1, 0)] = (np.eye(3) / s3)[:, :, None]
    C[(1, 1, 2)] = np.transpose(B, (1, 2, 0)) / s5
    C[(1, 2, 1)] = np.transpose(B, (1, 0, 2)) / s5
    C[(2, 1, 1)] = B / s5
    C[(2, 2, 0)] = (np.eye(5) / s5)[:, :, None]
    T = np.einsum('aij,bjk,cki->abc', B, B, B)
    C[(2, 2, 2)] = T / np.linalg.norm(T)
    return C

CG = cg_np()
PATH_LIST_O2 = O2_UVW + O2_UVU  # (i,j,k) in folded order


def support_pairs(path_ijk):
    d = {}
    for pi, (li, lj, lk) in enumerate(path_ijk):
        C = CG[(li, lj, lk)]
        for iloc in range(DIMS[li]):
            for jloc in range(DIMS[lj]):
                if np.any(np.abs(C[iloc, jloc, :]) > 1e-12):
                    d.setdefault(((li, iloc), (lj, jloc)), []).append((pi, iloc, jloc))
    return d


def build_mono_blocks_sym(path_ijk):
    d = support_pairs(path_ijk)
    blocks = {}
    for (I, J), lst in d.items():
        key = (min(I, J), max(I, J))
        swap = I > J
        for (pi, iloc, jloc) in lst:
            blocks.setdefault(key, []).append((pi, iloc, jloc, swap))
    return [(I, J, c) for (I, J), c in sorted(blocks.items())]


def build_mono_blocks(path_ijk):
    d = support_pairs(path_ijk)
    return [(I, J, [(pi, i, j, False) for (pi, i, j) in lst]) for (I, J), lst in sorted(d.items())]


def omega_for_block(path_ijk, weights, I, J, contribs):
    """[1024 (u-major: row = u*32+v, u from I), HID] kappa-major outputs."""
    Om = np.zeros((MUL * MUL, HID))
    for (pi, iloc, jloc, swap) in contribs:
        li, lj, lk = path_ijk[pi]
        W = weights[pi]
        C = CG[(li, lj, lk)]
        for kap in range(DIMS[lk]):
            c = C[iloc, jloc, kap]
            if abs(c) < 1e-12:
                continue
            c0 = LOFF[lk] + kap * 32
            Wm = W if not swap else np.transpose(W, (1, 0, 2))
            Om[:, c0:c0 + 32] += c * Wm.reshape(MUL * MUL, MUL)
    return Om


# ---------------------------------------------------------------------------
# static plan
# ---------------------------------------------------------------------------

class Plan:
    pass


def _groups_for_pair(mq, mm_):
    """Used output groups (0..17) in the combined q|msg space."""
    gs = set()
    for mask, base in ((mq, 0), (mm_, 9)):
        if mask is None:
            continue
        for g in range(9):
            if np.any(mask[:, g * 32:(g + 1) * 32]):
                gs.add(base + g)
    return gs


def _emissions_from_groups(gs, pos):
    """One emission list (shared by all 8 kchunks): contiguous position
    runs with gap tolerance, as (c0, c1) column ranges in position space."""
    used = sorted(pos[g] for g in gs)
    runs = [[used[0], used[0] + 1]]
    for g in used[1:]:
        if g - runs[-1][1] <= GAPTOL:
            runs[-1][1] = g + 1
        else:
            runs.append([g, g + 1])
    return [(ga * 32, gb * 32) for (ga, gb) in runs]


def _anneal_perm(gsets, seed=1, iters=250000, trials=3):
    import math, random
    def span_cost(perm):
        pos = [0] * 18
        for i, g in enumerate(perm):
            pos[g] = i
        s = 0
        for gs in gsets:
            ps = [pos[g] for g in gs]
            s += max(ps) - min(ps) + 1
        return s
    rng = random.Random(seed)
    best, bc = list(range(18)), span_cost(list(range(18)))
    for t in range(trials):
        cur = best[:] if t else list(range(18))
        cc = span_cost(cur)
        T = 6.0
        for it in range(iters):
            i = rng.randrange(18); j = rng.randrange(18)
            if i == j:
                continue
            cur[i], cur[j] = cur[j], cur[i]
            c = span_cost(cur)
            if c < cc or rng.random() < math.exp(min(0.0, (cc - c) / max(T, 1e-3))):
                cc = c
                if c < bc:
                    best, bc = cur[:], c
            else:
                cur[i], cur[j] = cur[j], cur[i]
            T *= 0.99998
    return best, bc


def _make_runs(order_key_blocks, span_key):
    """Group blocks (already sorted) into runs of consecutive span-comp
    indices with the same fixed comp, capped at RUNCAP."""
    runs = []
    cur = None
    for bi, (fixed_t, span_t) in enumerate(order_key_blocks):
        if (cur is not None and fixed_t == cur[0]
                and span_t == cur[1] + len(cur[2]) and len(cur[2]) < RUNCAP):
            cur[2].append(bi)
        else:
            cur = [fixed_t, span_t, [bi]]
            runs.append(cur)
    return [(f, s, idxs) for (f, s, idxs) in runs]


def build_plan():
    p = Plan()
    n3a = len(PATHS_FULL)
    ones_a = [np.ones((MUL, MUL, MUL)) for _ in PATHS_FULL]
    ones_o2 = [np.ones((MUL, MUL, MUL)) for _ in PATH_LIST_O2]

    # --- aa pass: blocks keyed (I<=J); monomial u-major (u in I replicated
    # via AX, v in J tiled via ATILE). Sorted by (tJ, tI); runs merge
    # consecutive tI for fixed J.
    aa_raw = build_mono_blocks_sym(PATHS_FULL + PATH_LIST_O2)
    aa = []
    for (I, J, contribs) in aa_raw:
        cq = [(pi, i, j, s) for (pi, i, j, s) in contribs if pi < n3a]
        cm = [(pi - n3a, i, j, s) for (pi, i, j, s) in contribs if pi >= n3a]
        mq = omega_for_block(PATHS_FULL, ones_a, I, J, cq) != 0 if cq else None
        mm_ = omega_for_block(PATH_LIST_O2, ones_o2, I, J, cm) != 0 if cm else None
        gs = _groups_for_pair(mq, mm_)
        aa.append((I, J, cq, cm, gs))
    aa.sort(key=lambda b: (TCOMP[b[1]], TCOMP[b[0]]))
    p.aa = aa
    p.aa_runs = _make_runs([(TCOMP[J], TCOMP[I]) for (I, J, cq, cm, gs) in aa],
                           span_key=None)

    # --- qa pass: blocks (I=q comp tiled via QTILE, J=a comp replicated via
    # AX); monomial v-major (omega rows transposed). Sorted by (tI, tJ).
    qa_raw = build_mono_blocks(PATHS_FULL)
    qa = []
    for (I, J, contribs) in qa_raw:
        mm_ = omega_for_block(PATHS_FULL, ones_a, I, J, contribs) != 0
        gs = _groups_for_pair(None, mm_)
        qa.append((I, J, contribs, gs))
    qa.sort(key=lambda b: (TCOMP[b[0]], TCOMP[b[1]]))
    p.qa = qa
    p.qa_runs = _make_runs([(TCOMP[I], TCOMP[J]) for (I, J, c, gs) in qa],
                           span_key=None)

    # --- output-column permutation: minimize emission spans
    gsets = [gs for (_, _, _, _, gs) in aa] + [gs for (_, _, _, gs) in qa]
    perm, span = _anneal_perm(gsets)
    p.perm = perm                       # position -> group
    p.pos = [0] * 18                    # group -> position
    for i, g in enumerate(perm):
        p.pos[g] = i
    # q extraction: positions of groups 0..8, as contiguous runs
    qpos = sorted(p.pos[g] for g in range(9))
    runs = [[qpos[0], qpos[0] + 1]]
    for x in qpos[1:]:
        if x == runs[-1][1]:
            runs[-1][1] = x + 1
        else:
            runs.append([x, x + 1])
    p.q_runs = [(a, b) for (a, b) in runs]
    # comp t9 -> col offset in q_bf (position-rank order)
    p.qcol = [32 * qpos.index(p.pos[t9]) for t9 in range(9)]

    # --- emissions + omega col offsets (separate aa / qa tensors)
    off = 0
    p.aa_emi = []
    for (I, J, cq, cm, gs) in aa:
        em = _emissions_from_groups(gs, p.pos)
        bk = []
        for kc in range(8):
            lst = []
            for (c0, c1) in em:
                lst.append((c0, c1, off)); off += c1 - c0
            bk.append(lst)
        p.aa_emi.append(bk)
    p.totc_aa = off
    off = 0
    p.qa_emi = []
    for (I, J, contribs, gs) in qa:
        em = _emissions_from_groups(gs, p.pos)
        bk = []
        for kc in range(8):
            lst = []
            for (c0, c1) in em:
                lst.append((c0, c1, off)); off += c1 - c0
            bk.append(lst)
        p.qa_emi.append(bk)
    p.totc_qa = off

    # --- engine assignment for node monomial runs (True -> gpsimd).
    # Only single-block runs: a 1-block gpsimd multiply (~2us) hides under
    # the preceding PE emissions; larger runs stall the in-order PE queue.
    p.run_gp = []
    gp_n = 0
    for (f, s, idxs) in p.aa_runs + p.qa_runs:
        take = len(idxs) == 1 and gp_n < GP_MAX_RUNS
        p.run_gp.append(take)
        if take:
            gp_n += 1
    return p


def _to_pos_space(plan, Full):
    """[1024, 576] combined kappa space -> position space."""
    out = np.zeros_like(Full)
    for p_ in range(18):
        g = plan.perm[p_]
        src_c = g * 32 if g < 9 else HID + (g - 9) * 32
        out[:, p_ * 32:(p_ + 1) * 32] = Full[:, src_c:src_c + 32]
    return out


def pack_omega(plan, Wfold):
    """omega_aa [128, totc_aa] (rows u-major), omega_qa [128, totc_qa]
    (rows v-major); columns in permuted position space."""
    W3a = Wfold['o3a_w']; Wo2 = Wfold['o2_w']; W3b = Wfold['o3b_w']
    om_aa = np.zeros((128, plan.totc_aa), np.float32)
    for bi, (I, J, cq, cm, gs) in enumerate(plan.aa):
        Om = np.zeros((MUL * MUL, 2 * HID))
        if cq:
            Om[:, :HID] = omega_for_block(PATHS_FULL, W3a, I, J, cq)
        if cm:
            Om[:, HID:] = omega_for_block(PATH_LIST_O2, Wo2, I, J, cm)
        Om = _to_pos_space(plan, Om)
        for kc in range(8):
            for (c0, c1, off) in plan.aa_emi[bi][kc]:
                om_aa[:, off:off + (c1 - c0)] = Om[kc * 128:(kc + 1) * 128, c0:c1]
    om_qa = np.zeros((128, plan.totc_qa), np.float32)
    for bi, (I, J, contribs, gs) in enumerate(plan.qa):
        Om = omega_for_block(PATHS_FULL, W3b, I, J, contribs)  # [1024, 288] u-major
        Omv = Om.reshape(32, 32, HID).transpose(1, 0, 2).reshape(1024, HID)
        Full = np.zeros((MUL * MUL, 2 * HID))
        Full[:, HID:] = Omv
        Full = _to_pos_space(plan, Full)
        for kc in range(8):
            for (c0, c1, off) in plan.qa_emi[bi][kc]:
                om_qa[:, off:off + (c1 - c0)] = Full[kc * 128:(kc + 1) * 128, c0:c1]
    return om_aa.astype(BF), om_qa.astype(BF)


def fold_weights(inp):
    f8 = np.float64
    mix_w = inp['mix_w'].astype(f8); comb_w = inp['comb_w'].astype(f8)
    M = np.einsum('olux,olxw->oluw', mix_w, comb_w) / MUL
    W1eff = np.einsum('lux,lxw->luw', inp['lin_o1'].astype(f8), M[0]) / SQM
    o2_w = []
    for pp, (i, j, k) in enumerate(O2_UVW):
        o2_w.append(np.einsum('uvx,xw->uvw', inp['o2_uvw'][pp].astype(f8) / MUL, M[1][k]))
    for pp, (i, j, k) in enumerate(O2_UVU):
        o2_w.append(np.einsum('uv,uw->uvw', inp['o2_uvu'][pp].astype(f8), M[1][k]) / SQM)
    o3a_w = [inp['o3a_uvw'][pp].astype(f8) / MUL for pp in range(len(PATHS_FULL))]
    o3b_w = [np.einsum('uvx,xw->uvw', inp['o3b_uvw'][pp].astype(f8) / MUL, M[2][k])
             for pp, (i, j, k) in enumerate(PATHS_FULL)]
    aw = inp['a_w'].astype(f8).reshape(RHID, 3, MUL, MUL)
    ab = inp['a_b'].astype(f8).reshape(3, MUL, MUL)
    scale = np.array([1.0 / np.sqrt(d) for d in DIMS]) / SQM
    aw = aw * scale[None, :, None, None]
    ab = ab * scale[:, None, None]
    A2 = np.transpose(aw, (0, 2, 1, 3)).reshape(RHID * MUL, 3 * MUL)
    B2 = np.transpose(ab, (1, 0, 2)).reshape(MUL, 3 * MUL)
    # fold rw3 into A2: At[(q,u), lw] = sum_r rw3[q,r] A2[(r,u), lw]
    rw3 = inp['r_w3'].astype(f8)
    At = np.einsum('qr,rum->qum', rw3, A2.reshape(RHID, MUL, 96)).reshape(RHID * MUL, 96)
    # fold b3 into B2
    b3 = inp['r_b3'].astype(f8)
    B2f = B2.copy()
    for u in range(MUL):
        B2f[u, :] += b3 @ A2[np.arange(RHID) * 32 + u, :]
    # c1 block-diagonal omegas per aT chunk (kappa-major rows/cols)
    omc1 = np.zeros((HID, HID))
    for l in range(3):
        for i in range(DIMS[l]):
            c = LOFF[l] + i * 32
            omc1[c:c + 32, c:c + 32] = W1eff[l]
    return dict(
        o3a_w=o3a_w, o2_w=o2_w, o3b_w=o3b_w,
        omc1=omc1, omself=inp['self_w'].astype(f8) / SQM,
        emb=inp['emb_w'].astype(f8) / SQM,
        At=At, B2f=B2f,
        r_w1=inp['r_w1'].astype(np.float32), r_b1=inp['r_b1'].astype(np.float32),
        r_w2=inp['r_w2'].astype(np.float32), r_b2=inp['r_b2'].astype(np.float32),
    )


def edge_tile_counts(inp):
    dst = np.asarray(inp['edge_index'][1]).astype(np.int64)
    dst_s = np.sort(dst)
    tiles = []
    for w in range(N_WIN):
        mx = 0
        for c in range(N_CORES):
            lo = c * NODES_PER_CORE
            nlo = lo + w * WIN
            nhi = min(lo + (w + 1) * WIN, lo + NODES_PER_CORE)
            a = np.searchsorted(dst_s, nlo); b = np.searchsorted(dst_s, nhi)
            mx = max(mx, b - a)
        tiles.append(int(np.ceil(mx / 128)))
    return tiles


def pack_edges(inp):
    src = np.asarray(inp['edge_index'][0]).astype(np.int64)
    dst = np.asarray(inp['edge_index'][1]).astype(np.int64)
    sh = np.asarray(inp['edge_sh'], dtype=np.float32)
    rad = np.asarray(inp['edge_radial_embedding'], dtype=np.float32)
    attr = np.asarray(inp['edge_attr'], dtype=np.float32)
    nf = np.asarray(inp['node_features'], dtype=np.float32)
    order = np.argsort(dst, kind='stable')
    dst_s = dst[order]
    cores = []
    for c in range(N_CORES):
        lo = c * NODES_PER_CORE
        rinT = np.zeros((24, E_PAD), np.float32)
        nfsT = np.zeros((MUL, E_PAD), np.float32)
        shdv = np.zeros((E_PAD, 12), np.float32)   # 0..8 sh, 9 dst_local, 10 valid
        for w in range(N_WIN):
            nlo = lo + w * WIN
            nhi = min(lo + (w + 1) * WIN, lo + NODES_PER_CORE)
            a = np.searchsorted(dst_s, nlo); b = np.searchsorted(dst_s, nhi)
            idx = order[a:b]
            n = b - a
            assert n <= ESLOT, f"window overflow {n}"
            s = w * ESLOT
            rinT[:8, s:s + n] = rad[idx].T
            rinT[8:, s:s + n] = attr[idx].T
            nfsT[:, s:s + n] = nf[src[idx]].T
            shdv[s:s + n, :9] = sh[idx]
            shdv[s:s + n, 9] = (dst[idx] - nlo).astype(np.float32)
            shdv[s:s + n, 10] = 1.0
        # shdv repacked partition-major: [128, N_WIN * 8 * 12]
        shdv_p = shdv.reshape(N_WIN * ET_PER_WIN, 128, 12).transpose(1, 0, 2).reshape(128, -1)
        nfT = np.zeros((MUL, N_WIN * WIN), BF)
        nfT[:, :NODES_PER_CORE] = nf[lo:lo + NODES_PER_CORE].T.astype(BF)
        cores.append(dict(rinT=rinT.astype(BF), nfsT=nfsT.astype(BF),
                          shdv=shdv_p, nfT=nfT))
    return cores


def ref_from_kap(x_kap):
    out = np.empty_like(x_kap)
    for l, d in enumerate(DIMS):
        blk = x_kap[:, LOFF[l]:LOFF[l] + 32 * d].reshape(-1, d, 32)
        out[:, LOFF[l]:LOFF[l] + 32 * d] = np.transpose(blk, (0, 2, 1)).reshape(-1, 32 * d)
    return out


# ---------------------------------------------------------------------------
# device kernel
# ---------------------------------------------------------------------------

_NC_CACHE = {}
LAST_RESULT = None


def build_nc(plan, tiles):
    import concourse.bass as bass
    import concourse.bacc as bacc
    import concourse.mybir as mybir
    import concourse.tile as tile

    f32 = mybir.dt.float32
    bf16 = mybir.dt.bfloat16
    AL = mybir.AluOpType
    AF = mybir.ActivationFunctionType

    nc = bacc.Bacc(None)
    P = 128

    # ---- dram parameters
    rinT_d = nc.declare_dram_parameter("rinT", [24, E_PAD], bf16, isOutput=False)
    nfsT_d = nc.declare_dram_parameter("nfsT", [32, E_PAD], bf16, isOutput=False)
    shdv_d = nc.declare_dram_parameter("shdv", [P, N_WIN * ET_PER_WIN * 12], f32, isOutput=False)
    nfT_d = nc.declare_dram_parameter("nfT", [32, N_WIN * WIN], bf16, isOutput=False)
    om_aa_d = nc.declare_dram_parameter("om_aa", [P, plan.totc_aa], bf16, isOutput=False)
    om_qa_d = nc.declare_dram_parameter("om_qa", [P, plan.totc_qa], bf16, isOutput=False)
    a2t_d = nc.declare_dram_parameter("a2t", [P, 16 * 96], bf16, isOutput=False)
    b2_d = nc.declare_dram_parameter("b2", [32, 96], bf16, isOutput=False)
    omc1_d = nc.declare_dram_parameter("omc1", [P, HID], bf16, isOutput=False)
    omself_d = nc.declare_dram_parameter("omself", [32, 32], bf16, isOutput=False)
    rw1_d = nc.declare_dram_parameter("rw1", [24, 64], bf16, isOutput=False)
    rw2_d = nc.declare_dram_parameter("rw2", [64, 64], bf16, isOutput=False)
    rb1_d = nc.declare_dram_parameter("rb1", [64, 1], f32, isOutput=False)
    rb2_d = nc.declare_dram_parameter("rb2", [64, 1], f32, isOutput=False)
    emb4_d = nc.declare_dram_parameter("emb4", [32, P], bf16, isOutput=False)
    iotab_d = nc.declare_dram_parameter("iotab", [P, P], bf16, isOutput=False)
    identb_d = nc.declare_dram_parameter("identb", [P, P], bf16, isOutput=False)
    rsel_d = nc.declare_dram_parameter("rsel", [P, 32 * P], bf16, isOutput=False)
    rt4_d = nc.declare_dram_parameter("rt4", [P, 4 * P], bf16, isOutput=False)
    r64_d = nc.declare_dram_parameter("r64", [64, 16 * P], bf16, isOutput=False)
    zer_d = nc.declare_dram_parameter("zer", [1, P], bf16, isOutput=False)
    zer2_d = nc.declare_dram_parameter("zer2", [1, 2 * HID], bf16, isOutput=False)
    out_d = nc.declare_dram_parameter("out", [N_WIN * WIN, 2 * HID], f32, isOutput=True)

    from contextlib import ExitStack
    with tile.TileContext(nc) as tc, ExitStack() as es:
        cst = es.enter_context(tc.tile_pool(name="cst", bufs=1))
        sb = es.enter_context(tc.tile_pool(name="sb", bufs=2))
        sb3 = es.enter_context(tc.tile_pool(name="sb3", bufs=3))
        sbpt = es.enter_context(tc.tile_pool(name="sbpt", bufs=3))
        nodep = es.enter_context(tc.tile_pool(name="nodep", bufs=1))
        pse = es.enter_context(tc.tile_pool(name="pse", bufs=1, space="PSUM"))
        psw = es.enter_context(tc.tile_pool(name="psw", bufs=2, space="PSUM"))
        psq = es.enter_context(tc.tile_pool(name="psq", bufs=1, space="PSUM"))
        psn = es.enter_context(tc.tile_pool(name="psn", bufs=2, space="PSUM"))
        psn2 = es.enter_context(tc.tile_pool(name="psn2", bufs=1, space="PSUM"))

        # ---- constants into SBUF
        om_aa = cst.tile([P, plan.totc_aa], bf16)
        nc.sync.dma_start(out=om_aa[:], in_=om_aa_d[:])
        om_qa = cst.tile([P, plan.totc_qa], bf16)
        nc.sync.dma_start(out=om_qa[:], in_=om_qa_d[:])
        a2t = cst.tile([P, 16 * 96], bf16); nc.sync.dma_start(out=a2t[:], in_=a2t_d[:])
        b2 = cst.tile([32, 96], bf16); nc.sync.dma_start(out=b2[:], in_=b2_d[:])
        omc1 = cst.tile([P, HID], bf16); nc.sync.dma_start(out=omc1[:], in_=omc1_d[:])
        omself = cst.tile([32, 32], bf16); nc.sync.dma_start(out=omself[:], in_=omself_d[:])
        rw1 = cst.tile([24, 64], bf16); nc.sync.dma_start(out=rw1[:], in_=rw1_d[:])
        rw2 = cst.tile([64, 64], bf16); nc.sync.dma_start(out=rw2[:], in_=rw2_d[:])
        rb1 = cst.tile([64, 1], f32); nc.sync.dma_start(out=rb1[:], in_=rb1_d[:])
        rb2 = cst.tile([64, 1], f32); nc.sync.dma_start(out=rb2[:], in_=rb2_d[:])
        emb4 = cst.tile([32, P], bf16); nc.sync.dma_start(out=emb4[:], in_=emb4_d[:])
        iotab = cst.tile([P, P], bf16); nc.sync.dma_start(out=iotab[:], in_=iotab_d[:])
        identb = cst.tile([P, P], bf16); nc.sync.dma_start(out=identb[:], in_=identb_d[:])
        rsel = cst.tile([P, 32 * P], bf16); nc.sync.dma_start(out=rsel[:], in_=rsel_d[:])
        rt4 = cst.tile([P, 4 * P], bf16); nc.sync.dma_start(out=rt4[:], in_=rt4_d[:])
        r64 = cst.tile([64, 16 * P], bf16); nc.sync.dma_start(out=r64[:], in_=r64_d[:])
        zer = cst.tile([1, P], bf16); nc.sync.dma_start(out=zer[:], in_=zer_d[:])
        zer2 = cst.tile([1, 2 * HID], bf16); nc.sync.dma_start(out=zer2[:], in_=zer2_d[:])
        nfT = cst.tile([32, N_WIN * WIN], bf16); nc.sync.dma_start(out=nfT[:], in_=nfT_d[:])

        # persistent node-phase SBUF arrays; AX/ATILE/aT/a_bf double-buffered
        # so window w+1's prelude can overlap window w's qa phase
        AXs = [nodep.tile([P, 9 * 1024], bf16, tag=f"AX{i}", name=f"AX{i}") for i in range(2)]
        ATILEs = [nodep.tile([P, 9 * P], bf16, tag=f"ATILE{i}", name=f"ATILE{i}") for i in range(2)]
        aTs = [nodep.tile([P, 3 * P], bf16, tag=f"aT{i}", name=f"aT{i}") for i in range(2)]
        a_bfs = [nodep.tile([P, HID], bf16, tag=f"abf{i}", name=f"abf{i}") for i in range(2)]
        QTILE = nodep.tile([P, 9 * P], bf16)
        qT = nodep.tile([P, 3 * P], bf16)
        for t in aTs:
            nc.gpsimd.memset(t[:], 0.0)
        nc.gpsimd.memset(qT[:], 0.0)
        q_bf = nodep.tile([P, HID], bf16)

        def edge_tile(st):
            j = st['j']; st['j'] += 1
            rin_w, nfs_w, shdv_w, wps = st['rin'], st['nfs'], st['shdv'], st['wps']
            shdv_t = shdv_w[:, j * 12:(j + 1) * 12]
            rin_t = rin_w[:, j * P:(j + 1) * P]
            nfs_t = nfs_w[:, j * P:(j + 1) * P]
            eb = pse.tile([P, 512], f32, tag="eb")
            l1p = eb[0:64, 0:128]
            l2p = eb[0:64, 128:256]
            htilp = eb[:, 256:384]
            mxp = eb[:, 384:480]

            # htil[p, e] = h[e, p%32] via pre-tiled emb4 (rows 0:32 are hT itself)
            nc.tensor.matmul(out=htilp, lhsT=emb4[:], rhs=nfs_t, start=True, stop=True)
            htil = sb.tile([P, P], bf16, tag="htil_sb")
            nc.scalar.copy(out=htil[:], in_=htilp)
            # radial MLP (feature-major)
            nc.tensor.matmul(out=l1p, lhsT=rw1[:], rhs=rin_t, start=True, stop=True)
            f1 = sb.tile([64, P], bf16, tag="f1")
            nc.scalar.activation(out=f1[:], in_=l1p, func=AF.Silu, bias=rb1[:], scale=1.0)
            nc.tensor.matmul(out=l2p, lhsT=rw2[:], rhs=f1[:], start=True, stop=True)
            f2 = sb.tile([64, P], bf16, tag="f2")
            nc.scalar.activation(out=f2[:], in_=l2p, func=AF.Silu, bias=rb2[:], scale=1.0)
            # monomials mT[(q,u), e]: 4 quarters of 512
            # (fx PSUM -> act evac bf16 -> DVE 2x multiply)
            mt = sb.tile([P, 2048], bf16, tag="mt")
            for h4 in range(4):
                fx = psn.tile([P, 512], f32, tag="axp")
                for g in range(4):
                    gg = h4 * 4 + g
                    nc.tensor.matmul(out=fx[:, g * P:(g + 1) * P],
                                     lhsT=r64[:, gg * P:(gg + 1) * P],
                                     rhs=f2[:], start=True, stop=True)
                nc.vector.tensor_tensor(
                    out=mt[:, h4 * 512:(h4 + 1) * 512].rearrange("p (c z) -> p c z", z=P),
                    in0=fx[:].rearrange("p (c z) -> p c z", z=P),
                    in1=htil[:][:, None, :].broadcast_to([P, 4, P]),
                    op=AL.mult)
            # mixed = mT @ At + h @ B2f  (PSUM accumulate)
            for c in range(16):
                nc.tensor.matmul(out=mxp, lhsT=mt[:, c * P:(c + 1) * P],
                                 rhs=a2t[:, c * 96:(c + 1) * 96],
                                 start=(c == 0), stop=False)
            nc.tensor.matmul(out=mxp, lhsT=htil[0:32, :], rhs=b2[:], start=False, stop=True)
            # messages (bf16) + valid col
            msgs = sb.tile([P, HID + 1], bf16, tag="msgs")
            for l, d in enumerate(DIMS):
                nc.vector.tensor_tensor(
                    out=msgs[:, LOFF[l]:LOFF[l] + 32 * d].rearrange("p (i u) -> p i u", u=32),
                    in0=shdv_t[:, SOFF[l]:SOFF[l] + d][:, :, None].broadcast_to([P, d, 32]),
                    in1=mxp[:, l * 32:(l + 1) * 32][:, None, :].broadcast_to([P, d, 32]),
                    op=AL.mult)
            nc.vector.tensor_copy(out=msgs[:, HID:HID + 1], in_=shdv_t[:, 10:11])
            # one-hot S via tensor_scalar is_equal (2x bf16 mode)
            S = sb.tile([P, P], bf16, tag="S")
            nc.gpsimd.tensor_scalar(out=S[:], in0=iotab[:], scalar1=shdv_t[:, 9:10],
                                    scalar2=None, op0=AL.is_equal)
            nc.tensor.matmul(out=wps, lhsT=S[:], rhs=msgs[:],
                             start=(j == 0), stop=(j == st['ntiles'] - 1))

        def build_rep(srcT, dstAX, dstTILE, evac_eng):
            """srcT: [128, 384] bf16 feature-major chunks. Fills dstAX
            [128, 9*1024] (4-row replicate x32) and dstTILE [128, 9*128]
            (32-row tile x4) via selection matmuls + batched PSUM evacs."""
            for t9 in range(9):
                cI = LOFF[COMPS[t9][0]] + COMPS[t9][1] * 32
                ch, c0 = cI // P, cI % P
                for half in range(2):
                    axp = psn.tile([P, 512], f32, tag="axp")
                    for g in range(4):
                        gg = half * 4 + g
                        o = (c0 + 4 * gg) // 4
                        nc.tensor.matmul(out=axp[:, g * P:(g + 1) * P],
                                         lhsT=rsel[:, o * P:(o + 1) * P],
                                         rhs=srcT[:, ch * P:(ch + 1) * P],
                                         start=True, stop=True)
                    evac_eng.copy(out=dstAX[:, t9 * 1024 + half * 512: t9 * 1024 + (half + 1) * 512],
                                  in_=axp[:])
            for pair in range(5):
                t9a = pair * 2
                axp = psn.tile([P, 512], f32, tag="axp")
                npair = 1 if t9a + 1 >= 9 else 2
                for k in range(npair):
                    t9 = t9a + k
                    cI = LOFF[COMPS[t9][0]] + COMPS[t9][1] * 32
                    ch, c0 = cI // P, cI % P
                    nc.tensor.matmul(out=axp[:, k * P:(k + 1) * P],
                                     lhsT=rt4[:, (c0 // 32) * P:(c0 // 32 + 1) * P],
                                     rhs=srcT[:, ch * P:(ch + 1) * P],
                                     start=True, stop=True)
                evac_eng.copy(out=dstTILE[:, t9a * P:(t9a + npair) * P],
                              in_=axp[:, 0:npair * P])

        def transpose3(src_bf, dstT, evac_eng):
            tpa = psn2.tile([P, 512], bf16, tag="tpa")
            nc.tensor.transpose(out=tpa[:, 0:P], in_=src_bf[:, 0:P], identity=identb[:])
            nc.tensor.transpose(out=tpa[:, P:2 * P], in_=src_bf[:, P:2 * P], identity=identb[:])
            nc.tensor.transpose(out=tpa[0:32, 2 * P:3 * P], in_=src_bf[:, 2 * P:HID], identity=identb[:])
            evac_eng.copy(out=dstT[:, 0:2 * P], in_=tpa[:, 0:2 * P])
            evac_eng.copy(out=dstT[0:32, 2 * P:3 * P], in_=tpa[0:32, 2 * P:3 * P])

        def mono_runs(AX, runs, blocks, emi, tile_src, om, qm, run_gp_off, pump):
            """Monomial runs + emissions. tile_src: ATILE (aa) or QTILE (qa)."""
            for ri, (ft, st, idxs) in enumerate(runs):
                n = len(idxs)
                PT = sbpt.tile([P, RUNCAP * 1024], bf16, tag="PT")
                eng = nc.gpsimd if plan.run_gp[run_gp_off + ri] else nc.vector
                eng.tensor_tensor(
                    out=PT[:, 0:n * 1024].rearrange("p (c z) -> p c z", z=P),
                    in0=AX[:, st * 1024:(st + n) * 1024].rearrange("p (c z) -> p c z", z=P),
                    in1=tile_src[:, ft * P:(ft + 1) * P][:, None, :].broadcast_to([P, 8 * n, P]),
                    op=AL.mult)
                for bl, bi in enumerate(idxs):
                    for kc in range(8):
                        for (c0, c1, off) in emi[bi][kc]:
                            nc.tensor.matmul(out=qm[:, c0:c1],
                                             lhsT=PT[:, (bl * 8 + kc) * P:(bl * 8 + kc + 1) * P],
                                             rhs=om[:, off:off + (c1 - c0)],
                                             start=False, stop=False,
                                             skip_group_check=True)
                pump()

        # ---------------- main loop (edge w+1 pipelined into node w) ----
        def begin_edge(w):
            rin_w = sb.tile([24, ESLOT], bf16, tag="rinw")
            nc.sync.dma_start(out=rin_w[:], in_=rinT_d[:, w * ESLOT:(w + 1) * ESLOT])
            nfs_w = sb.tile([32, ESLOT], bf16, tag="nfsw")
            nc.sync.dma_start(out=nfs_w[:], in_=nfsT_d[:, w * ESLOT:(w + 1) * ESLOT])
            shdv_w = sb.tile([P, ET_PER_WIN * 12], f32, tag="shdvw")
            nc.sync.dma_start(out=shdv_w[:], in_=shdv_d[:, w * ET_PER_WIN * 12:(w + 1) * ET_PER_WIN * 12])
            wps = psw.tile([P, HID + 1], f32, tag="wps")
            return dict(rin=rin_w, nfs=nfs_w, shdv=shdv_w, wps=wps, j=0,
                        ntiles=tiles[w])

        def prelude(w, wps):
            """Extract a(w) from wps and build aT/AX/ATILE into buffer w%2."""
            buf = w % 2
            a_bf, aT, AX, ATILE = a_bfs[buf], aTs[buf], AXs[buf], ATILEs[buf]
            cnt = sb.tile([P, 1], f32, tag="cnt")
            nc.vector.tensor_scalar_max(out=cnt[:], in0=wps[:, HID:HID + 1], scalar1=1.0)
            rec = sb.tile([P, 1], f32, tag="rec")
            nc.vector.reciprocal(out=rec[:], in_=cnt[:])
            nc.scalar.activation(out=a_bf[:], in_=wps[:, 0:HID], func=AF.Copy, scale=rec[:])
            transpose3(a_bf, aT, nc.scalar)
            build_rep(aT, AX, ATILE, nc.scalar)

        st = begin_edge(0)
        for _ in range(tiles[0]):
            edge_tile(st)
        prelude(0, st['wps'])
        for w in range(N_WIN):
            buf = w % 2
            a_bf, aT, AX, ATILE = a_bfs[buf], aTs[buf], AXs[buf], ATILEs[buf]
            nxt = begin_edge(w + 1) if w + 1 < N_WIN else None
            pump_state = dict(cnt=0, prelude_done=False)

            def pump():
                # issue one pipelined edge tile of window w+1 every 3rd run;
                # once all tiles are in, issue window w+1's prelude
                pump_state['cnt'] += 1
                if nxt is None:
                    return
                if pump_state['cnt'] % 4 == 0 and nxt['j'] < nxt['ntiles']:
                    edge_tile(nxt)
                elif nxt['j'] >= nxt['ntiles'] and not pump_state['prelude_done']:
                    prelude(w + 1, nxt['wps'])
                    pump_state['prelude_done'] = True

            qm = psq.tile([P, 2 * HID], f32, tag="qm")
            nc.tensor.matmul(out=qm[:, 0:512], lhsT=zer[:], rhs=zer2[:, 0:512], start=True, stop=False)
            nc.tensor.matmul(out=qm[:, 512:576], lhsT=zer[:], rhs=zer2[:, 512:576], start=True, stop=False)

            mono_runs(AX, plan.aa_runs, plan.aa, plan.aa_emi, ATILE, om_aa, qm, 0, pump)

            # q -> bf16 (position-rank layout), transpose, tile
            qoff = 0
            for (pa, pb) in plan.q_runs:
                w32 = (pb - pa) * 32
                nc.scalar.copy(out=q_bf[:, qoff:qoff + w32], in_=qm[:, pa * 32:pb * 32])
                qoff += w32
            transpose3(q_bf, qT, nc.scalar)
            for pair in range(5):
                t9a = pair * 2
                axp = psn.tile([P, 512], f32, tag="axp")
                npair = 1 if t9a + 1 >= 9 else 2
                for k in range(npair):
                    t9 = t9a + k
                    qc = plan.qcol[t9]
                    ch, c0 = qc // P, qc % P
                    nc.tensor.matmul(out=axp[:, k * P:(k + 1) * P],
                                     lhsT=rt4[:, (c0 // 32) * P:(c0 // 32 + 1) * P],
                                     rhs=qT[:, ch * P:(ch + 1) * P],
                                     start=True, stop=True)
                nc.scalar.copy(out=QTILE[:, t9a * P:(t9a + npair) * P],
                               in_=axp[:, 0:npair * P])
            mono_runs(AX, plan.qa_runs, plan.qa, plan.qa_emi, QTILE, om_qa, qm,
                      len(plan.aa_runs), pump)
            # flush any unissued pipelined work for window w+1
            while nxt is not None and nxt['j'] < nxt['ntiles']:
                edge_tile(nxt)
            if nxt is not None and not pump_state['prelude_done']:
                prelude(w + 1, nxt['wps'])

            # c1: msg group g += aT-chunk @ omc1 32-col slice (block-diag linear)
            for g in range(9):
                posc = plan.pos[9 + g] * 32
                gcol = g * 32
                ch = gcol // P
                if ch < 2:
                    nc.tensor.matmul(out=qm[:, posc:posc + 32],
                                     lhsT=aT[:, ch * P:(ch + 1) * P],
                                     rhs=omc1[:, gcol:gcol + 32], start=False,
                                     stop=False, skip_group_check=True)
                else:
                    nc.tensor.matmul(out=qm[:, posc:posc + 32],
                                     lhsT=aT[0:32, 2 * P:3 * P],
                                     rhs=omc1[0:32, gcol:gcol + 32], start=False,
                                     stop=False, skip_group_check=True)
            # self connection (scalar msg group)
            nc.tensor.matmul(out=qm[:, plan.pos[9] * 32:plan.pos[9] * 32 + 32],
                             lhsT=nfT[:, w * P:(w + 1) * P],
                             rhs=omself[:], start=False, stop=True,
                             skip_group_check=True)
            out_sb = sb.tile([P, 2 * HID], f32, tag="outsb")
            nc.scalar.copy(out=out_sb[:], in_=qm[:, 0:2 * HID])
            nc.sync.dma_start(out=out_d[w * P:(w + 1) * P, :], in_=out_sb[:])
            st = nxt

    nc.finalize()
    return nc


def _get_nc(plan, tiles):
    if 'nc' not in _NC_CACHE:
        _NC_CACHE['nc'] = build_nc(plan, tiles)
    return _NC_CACHE['nc']


def kernel(**inputs):
    global LAST_RESULT
    from concourse.bass_utils import run_bass_kernel_spmd

    inp = {k: np.asarray(v) for k, v in inputs.items()}
    plan = build_plan()
    W = fold_weights(inp)
    om_aa, om_qa = pack_omega(plan, W)

    # At packed [128, 16*96]: chunk c rows [c*128:(c+1)*128] -> cols [c*96:(c+1)*96]
    At = W['At'].astype(np.float32)
    a2p = np.zeros((128, 16 * 96), np.float32)
    for c in range(16):
        a2p[:, c * 96:(c + 1) * 96] = At[c * 128:(c + 1) * 128, :]
    # omc1 packed [128, 288]
    omc1 = W['omc1']
    omc1p = np.zeros((128, HID), np.float32)
    omc1p[:, 0:128] = omc1[0:128, 0:128]
    omc1p[:, 128:256] = omc1[128:256, 128:256]
    omc1p[0:32, 256:288] = omc1[256:288, 256:288]

    iotab = np.broadcast_to(np.arange(128, dtype=np.float32)[None, :], (128, 128)).astype(BF).copy()
    identb = np.eye(128, dtype=np.float32).astype(BF)
    pidx = np.arange(128)
    rsel = np.zeros((128, 32, 128), np.float32)
    for t in range(32):
        rsel[4 * t + pidx // 32, t, pidx] = 1.0
    rt4 = np.zeros((128, 4, 128), np.float32)
    for v in range(4):
        rt4[32 * v + pidx % 32, v, pidx] = 1.0
    r64 = np.zeros((64, 16, 128), np.float32)
    for g in range(16):
        r64[4 * g + pidx // 32, g, pidx] = 1.0

    shared = dict(
        om_aa=om_aa, om_qa=om_qa,
        a2t=a2p.astype(BF), b2=W['B2f'].astype(np.float32).astype(BF),
        omc1=omc1p.astype(BF), omself=W['omself'].astype(np.float32).astype(BF),
        rw1=W['r_w1'].astype(BF), rw2=W['r_w2'].astype(BF),
        rb1=W['r_b1'].reshape(64, 1), rb2=W['r_b2'].reshape(64, 1),
        emb4=np.tile(W['emb'].astype(np.float32), (1, 4)).astype(BF),
        iotab=iotab, identb=identb,
        rsel=rsel.reshape(128, -1).astype(BF),
        rt4=rt4.reshape(128, -1).astype(BF),
        r64=r64.reshape(64, -1).astype(BF),
        zer=np.zeros((1, 128), BF), zer2=np.zeros((1, 2 * HID), BF),
    )
    cores = pack_edges(inp)
    in_maps = []
    for c in range(N_CORES):
        m = dict(shared)
        m.update(rinT=cores[c]['rinT'], nfsT=cores[c]['nfsT'],
                 shdv=cores[c]['shdv'], nfT=cores[c]['nfT'])
        in_maps.append(m)

    nc = _get_nc(plan, edge_tile_counts(inp))
    res = run_bass_kernel_spmd(nc, in_maps, core_ids=list(range(N_CORES)))
    LAST_RESULT = res
    outs = [res.results[c]['out'][:NODES_PER_CORE] for c in range(N_CORES)]
    dump = np.concatenate(outs, axis=0).astype(np.float32)
    out_kap = np.empty((N_NODES, HID), np.float32)
    for g in range(9):
        pc = plan.pos[9 + g] * 32
        out_kap[:, g * 32:(g + 1) * 32] = dump[:, pc:pc + 32]
    return ref_from_kap(out_kap)


if __name__ == "__main__":
    plan = build_plan()
    print(f"aa blocks: {len(plan.aa)} in {len(plan.aa_runs)} runs; "
          f"qa blocks: {len(plan.qa)} in {len(plan.qa_runs)} runs")
    print(f"omega cols: aa {plan.totc_aa} qa {plan.totc_qa} "
          f"({(plan.totc_aa + plan.totc_qa) * 128 * 2 / 1e6:.1f} MB bf16)")
    n_emi = sum(len(l) for bk in plan.aa_emi + plan.qa_emi for l in bk)
    print(f"emissions/window: {n_emi}; perm {plan.perm}; q_runs {plan.q_runs}")
    ngp = sum(plan.run_gp)
    print(f"gpsimd runs: {ngp}/{len(plan.run_gp)}")
